# revision 32
# baseline (speedup 1.0000x reference)
"""Bidirectional Mamba block on 8 Trainium2 NeuronCores.

Sharding: core c in 0..7 handles (branch = c // 4, batch = c % 4) where
branch 0 = fwd, branch 1 = bwd (bwd runs on the time-flipped input; flip
is applied host-side before dispatch and on the partial output after).

Per-core device pipeline (one full mamba branch for one batch element):
  LN -> in_proj (PE, bf16, ln-gamma folded into W) -> causal depthwise
  conv (PE, diag matmuls) -> silu -> x_proj (PE) -> dt_proj + softplus
  (exp/ln) -> selective scan: u = dx*B via one broadcast multiply,
  per-state in-place tensor_tensor_scan, hc = h*C in one multiply
  (split DVE/Pool by k), y = sum_s hC_s via PE identity-matmul PSUM
  accumulation seeded with diag(D)*xi -> gate with resident silu(z)
  -> fused (merge_half @ out_w) matmul -> partial output [d_model, L].

Host combines: out = x + part_fwd^T + flip(part_bwd^T) + merge_b.
"""

import math
import os
import sys
from contextlib import ExitStack

import numpy as np

sys.path.insert(0, "/opt/trn_rl_repo")
sys.path.insert(0, "/opt/trn_rl_repo/concourse")

import ml_dtypes  # noqa: E402

import concourse.bass as bass  # noqa: E402
import concourse.tile as tile  # noqa: E402
from concourse import bacc, mybir  # noqa: E402
from concourse.bass_utils import run_bass_kernel_spmd  # noqa: E402
from concourse.masks import make_identity  # noqa: E402

FP32 = mybir.dt.float32
BF16 = mybir.dt.bfloat16
OP = mybir.AluOpType
ACTF = mybir.ActivationFunctionType
BF16_NP = ml_dtypes.bfloat16

# ---- balance knobs (tuned against TimelineSim) ----
S_POOL = 5         # states whose u-mul/hc-mul run on gpsimd (rest DVE)
LN_ON_POOL = False  # LayerNorm apply (sub/mul) on gpsimd
DX_ON_POOL = False


class Cfg:
    def __init__(self, L=2048, DM=1024, DI=2048, DS=16, DTR=64, DC=4, TC=512):
        self.L = L      # sequence length
        self.DM = DM    # d_model
        self.DI = DI    # d_inner
        self.DS = DS    # d_state
        self.DTR = DTR  # dt_rank
        self.DC = DC    # d_conv
        self.TC = TC    # time chunk
        self.P = 128
        self.NCH = L // TC          # time chunks
        self.NDH = DI // self.P     # d_inner 128-tiles
        self.NDM = DM // self.P     # d_model 128-tiles
        self.NLT = L // self.P      # L 128-tiles (for LN stats)
        assert L % TC == 0 and DI % 128 == 0 and DM % 128 == 0 and L % 128 == 0
        assert DTR <= 128 and DTR + 2 * DS <= 128


FULL = Cfg()


def build_program(cfg: Cfg, num_devices: int = 8):
    """Build the (shared-across-cores) Bass program."""
    nc = bacc.Bacc(
        "TRN2", target_bir_lowering=False, debug=False, num_devices=num_devices
    )
    P, L = cfg.P, cfg.L

    def ext_in(name, shape, dt=FP32):
        return nc.dram_tensor(name, shape, dt, kind="ExternalInput")

    io = {
        # activations
        "x_ld": ext_in("x_ld", [L, cfg.DM]),          # [L, d_model] fp32
        "x_dl": ext_in("x_dl", [cfg.DM, L]),          # transposed   fp32
        # weights (pre-transposed / pre-cast host side)
        "in_w_pk": ext_in("in_w_pk", [P, 2 * (cfg.DI // P) * cfg.DM], BF16),
        "in_b": ext_in("in_b", [2 * cfg.DI, 1]),      # in_w @ ln_b
        "conv_dg_pk": ext_in(
            "conv_dg_pk", [P, (cfg.DI // P) * cfg.DC * P], BF16),
        "conv_b": ext_in("conv_b", [cfg.DI, 1]),
        "xproj_wT": ext_in("xproj_wT", [cfg.DI, cfg.DTR + 2 * cfg.DS], BF16),
        "dt_wT": ext_in("dt_wT", [cfg.DTR, cfg.DI], BF16),
        "dt_b": ext_in("dt_b", [cfg.DI, 1]),
        "A_neg": ext_in("A_neg", [cfg.DI, cfg.DS]),   # -exp(A_log) fp32
        "dD_pk": ext_in("dD_pk", [P, (cfg.DI // P) * P], BF16),  # diag(D)
        "w_comb_pk": ext_in("w_comb_pk", [P, (cfg.DM // P) * cfg.DI], BF16),
    }
    out = nc.dram_tensor("part_out", [cfg.DM, L], FP32, kind="ExternalOutput")
    # internal DRAM scratch
    scratch = {
        "mu_d": [nc.dram_tensor(f"mu_d{c}", [cfg.TC, 1], BF16)
                 for c in range(cfg.NCH)],
        "rstd_d": [nc.dram_tensor(f"rstd_d{c}", [cfg.TC, 1], BF16)
                   for c in range(cfg.NCH)],
        "bc_d": [nc.dram_tensor(f"bc_d{j}", [2 * cfg.DS, cfg.TC], BF16)
                 for j in range(2)],
    }

    with tile.TileContext(nc) as tc:
        with ExitStack() as ctx:
            _body(ctx, tc, cfg, io, out, scratch)
    nc.compile()
    return nc


def _body(ctx, tc, cfg, io, out_d, scratch):
    nc = tc.nc
    P, L, TC, DS, DC = cfg.P, cfg.L, cfg.TC, cfg.DS, cfg.DC
    NCH, NDH, NDM = cfg.NCH, cfg.NDH, cfg.NDM
    NLT, DTR = cfg.NLT, cfg.DTR
    CW = TC + DC - 1  # conv input window per chunk in the xz store
    NPJ = DTR + 2 * DS
    mu_d, rstd_d, bc_d = scratch["mu_d"], scratch["rstd_d"], scratch["bc_d"]

    # ---------------- persistent pools / tiles ----------------
    const_p = ctx.enter_context(tc.tile_pool(name="const", bufs=1))
    big_p = ctx.enter_context(tc.tile_pool(name="big", bufs=1))

    ident = const_p.tile([P, P], BF16, tag="ident")
    make_identity(nc, ident[:])

    # small per-channel columns packed into one tile:
    # [NDH conv_b][NDH dt_b][2*NDH in_b][1 eps]
    ncc = 4 * NDH + 1
    cols = const_p.tile([P, ncc], FP32, tag="cols")
    o_cb, o_db, o_ib = 0, NDH, 2 * NDH
    o_eps = 4 * NDH
    conv_b_c = lambda k: cols[:, o_cb + k:o_cb + k + 1]
    dt_b_c = lambda k: cols[:, o_db + k:o_db + k + 1]
    in_b_c = lambda m: cols[:, o_ib + m:o_ib + m + 1]
    eps_c = cols[:, o_eps:o_eps + 1]
    nc.vector.memset(eps_c, 1e-5)
    for k in range(NDH):
        r = slice(k * P, (k + 1) * P)
        nc.sync.dma_start(conv_b_c(k), io["conv_b"][r, :])
        nc.sync.dma_start(dt_b_c(k), io["dt_b"][r, :])
    for m in range(2 * NDH):
        r = slice(m * P, (m + 1) * P)
        nc.sync.dma_start(in_b_c(m), io["in_b"][r, :])

    # ---------------- phase 1: LayerNorm statistics ----------------
    # Batched by activation function so the ACT table never ping-pongs:
    # per-tile Square+accum, then batched Ln / Exp. Emitted in two groups
    # so chunk-0 prep can start as soon as its own stats exist.
    ln_p = ctx.enter_context(tc.tile_pool(name="ln", bufs=1))
    lns_p = ctx.enter_context(tc.tile_pool(name="lns", bufs=1))
    mu_all = lns_p.tile([P, NLT], BF16, tag="mu_all")
    ss_all = lns_p.tile([P, NLT], FP32, tag="ss_all")
    lv_all = lns_p.tile([P, NLT], FP32, tag="lv_all")
    rstd_all = lns_p.tile([P, NLT], BF16, tag="rstd_all")

    def emit_ln_stats(lts):
        for lt in lts:
            r = slice(lt * P, (lt + 1) * P)
            xt = ln_p.tile([P, cfg.DM], FP32, tag="x")
            nc.scalar.dma_start(xt[:], io["x_ld"][r, :])
            s1 = ln_p.tile([P, 1], FP32, tag="s1")
            nc.vector.reduce_sum(s1[:], xt[:], axis=mybir.AxisListType.X)
            negmu = ln_p.tile([P, 1], FP32, tag="negmu")
            nc.scalar.mul(negmu[:], s1[:], -1.0 / cfg.DM)
            nc.scalar.mul(mu_all[:, lt:lt + 1], s1[:], 1.0 / cfg.DM)
            sq = ln_p.tile([P, cfg.DM], BF16, tag="sq")
            nc.scalar.activation(sq[:], xt[:], ACTF.Square, bias=negmu[:],
                                 scale=1.0, accum_out=ss_all[:, lt:lt + 1])
        # rstd = exp(-0.5 * ln(var + eps)); keeps ACT in the exp/ln table
        ls = slice(lts[0], lts[-1] + 1)
        nc.scalar.activation(lv_all[:, ls], ss_all[:, ls], ACTF.Ln, bias=eps_c,
                             scale=1.0 / cfg.DM)
        nc.scalar.activation(rstd_all[:, ls], lv_all[:, ls], ACTF.Exp,
                             scale=-0.5)
        for lt in lts:
            cix = (lt * P) // TC
            roff = lt * P - cix * TC
            nc.sync.dma_start(mu_d[cix][roff:roff + P, :], mu_all[:, lt:lt + 1])
            nc.sync.dma_start(rstd_d[cix][roff:roff + P, :],
                              rstd_all[:, lt:lt + 1])

    emit_ln_stats(list(range(TC // P)))  # chunk 0's stats first

    a_sb = const_p.tile([P, NDH * DS], FP32, tag="aneg")
    for k in range(NDH):
        nc.sync.dma_start(a_sb[:, k * DS:(k + 1) * DS],
                          io["A_neg"][k * P:(k + 1) * P, :])

    # x_proj / dt_proj weights resident, bf16
    xprj_sb = const_p.tile([P, NDH * NPJ], BF16, tag="xprj")
    for k in range(NDH):
        nc.sync.dma_start(
            xprj_sb[:, k * NPJ:(k + 1) * NPJ], io["xproj_wT"][k * P:(k + 1) * P, :]
        )
    dtw_sb = const_p.tile([DTR, cfg.DI], BF16, tag="dtw")
    nc.sync.dma_start(dtw_sb[:], io["dt_wT"][:, :])
    dD_sb = const_p.tile([P, NDH * P], BF16, tag="dD")
    nc.sync.dma_start(dD_sb[:], io["dD_pk"][:, :])

    # ---------------- persistent chunk-state tiles ----------------
    xz_xi = big_p.tile([P, NDH * CW], BF16, tag="xz_xi")
    xiT = big_p.tile([P, NDH * TC], BF16, tag="xiT")
    delta = big_p.tile([P, NDH * TC], BF16, tag="delta")
    z_sb = big_p.tile([P, NDH * TC], BF16, tag="z_sb")
    b_big = big_p.tile([P, DS, TC], BF16, tag="b_big")
    c_big = big_p.tile([P, DS, TC], BF16, tag="c_big")
    u_big0 = big_p.tile([P, DS, TC], BF16, tag="u_big0")
    u_big1 = big_p.tile([P, DS, TC], BF16, tag="u_big1")
    u_big = [u_big0, u_big1]
    ygate = big_p.tile([P, NDH * TC], BF16, tag="ygate")
    carry = big_p.tile([P, NDH * DS], FP32, tag="carry")
    xnT = big_p.tile([P, NDM * TC], BF16, tag="xnT")

    for k in range(NDH):  # zero the conv left-pad for chunk 0
        nc.vector.memset(xz_xi[:, k * CW:k * CW + DC - 1], 0.0)

    wi_p = ctx.enter_context(tc.tile_pool(name="wi", bufs=3))
    wo_p = ctx.enter_context(tc.tile_pool(name="wo", bufs=2))
    mm_ps = ctx.enter_context(
        tc.tile_pool(name="mmps", bufs=3, space=bass.MemorySpace.PSUM))
    y_ps_p = ctx.enter_context(
        tc.tile_pool(name="yps", bufs=2, space=bass.MemorySpace.PSUM))
    xp_ps = ctx.enter_context(
        tc.tile_pool(name="xpps", bufs=1, space=bass.MemorySpace.PSUM))
    o_ps_p = ctx.enter_context(
        tc.tile_pool(name="ops", bufs=2, space=bass.MemorySpace.PSUM))
    sc_p = ctx.enter_context(tc.tile_pool(name="scan", bufs=3))
    t16_p = ctx.enter_context(tc.tile_pool(name="t16", bufs=2))
    ev_p = ctx.enter_context(tc.tile_pool(name="evac", bufs=2))
    ov_p = ctx.enter_context(tc.tile_pool(name="oev", bufs=1))
    bat_p = ctx.enter_context(tc.tile_pool(name="bat", bufs=1))
    lnc_p = ctx.enter_context(tc.tile_pool(name="lnc", bufs=1))

    ln_eng = nc.gpsimd if LN_ON_POOL else nc.vector
    dx_eng = nc.gpsimd if DX_ON_POOL else nc.vector

    def emit_ln_apply(ch):
        tsl = slice(ch * TC, (ch + 1) * TC)
        mus = lnc_p.tile([P, 2 * TC], BF16, tag="mus")
        nc.sync.dma_start(
            mus[:, 0:TC],
            mu_d[ch].ap().rearrange("l one -> one l")[0:1, :].partition_broadcast(P))
        nc.sync.dma_start(
            mus[:, TC:2 * TC],
            rstd_d[ch].ap().rearrange("l one -> one l")[0:1, :].partition_broadcast(P))
        for k in range(NDM):
            xtT = ev_p.tile([P, TC], FP32, tag="lnx")
            nc.sync.dma_start(xtT[:], io["x_dl"][k * P:(k + 1) * P, tsl])
            ln_eng.tensor_sub(xtT[:], xtT[:], mus[:, 0:TC])
            ln_eng.tensor_mul(xnT[:, k * TC:(k + 1) * TC], xtT[:],
                              mus[:, TC:2 * TC])

    def emit_in_proj_xi(ch, m):
        wt = wi_p.tile([P, NDM * P], BF16, tag="w_in")
        nc.sync.dma_start(
            wt[:], io["in_w_pk"][:, m * NDM * P:(m + 1) * NDM * P])
        ps = mm_ps.tile([P, TC], FP32, tag="mm")
        for k in range(NDM):
            nc.tensor.matmul(ps[:], wt[:, k * P:(k + 1) * P],
                             xnT[:, k * TC:(k + 1) * TC],
                             start=(k == 0), stop=(k == NDM - 1))
        nc.scalar.activation(xz_xi[:, m * CW + DC - 1:m * CW + DC - 1 + TC],
                             ps[:], ACTF.Identity, bias=in_b_c(m))

    def emit_in_proj_z(ch, m):
        # m in [0, NDH): z-half tile; direct Silu evac into z_sb
        wt = wi_p.tile([P, NDM * P], BF16, tag="w_in")
        nc.sync.dma_start(
            wt[:], io["in_w_pk"][:, (NDH + m) * NDM * P:(NDH + m + 1) * NDM * P])
        ps = mm_ps.tile([P, TC], FP32, tag="mm")
        for k in range(NDM):
            nc.tensor.matmul(ps[:], wt[:, k * P:(k + 1) * P],
                             xnT[:, k * TC:(k + 1) * TC],
                             start=(k == 0), stop=(k == NDM - 1))
        nc.scalar.activation(z_sb[:, m * TC:(m + 1) * TC], ps[:],
                             ACTF.Silu, bias=in_b_c(NDH + m))

    def emit_conv(ch, k):
        # matmuls + raw (Identity+bias) evac into xiT; silu applied later
        dg = wi_p.tile([P, DC * P], BF16, tag="w_dg")
        nc.sync.dma_start(
            dg[:], io["conv_dg_pk"][:, k * DC * P:(k + 1) * DC * P])
        ps = mm_ps.tile([P, TC], FP32, tag="mm")
        for t in range(DC):
            nc.tensor.matmul(
                ps[:], dg[:, t * P:(t + 1) * P],
                xz_xi[:, k * CW + t:k * CW + t + TC],
                start=(t == 0), stop=(t == DC - 1))
        nc.scalar.activation(xiT[:, k * TC:(k + 1) * TC], ps[:],
                             ACTF.Identity, bias=conv_b_c(k))
        if ch + 1 < NCH:  # carry last DC-1 input cols for the next chunk
            nc.vector.tensor_copy(xz_xi[:, k * CW:k * CW + DC - 1],
                                  xz_xi[:, k * CW + TC:k * CW + TC + DC - 1])

    def emit_xi_silu(ch, gs=(0, 1, 2, 3)):
        # in-place Silu over xiT, 4-tile-wide ops
        G = 4
        v = xiT[:].rearrange("p (kk t) -> p kk t", t=TC)
        for g in gs:
            nc.scalar.activation(v[:, g * G:(g + 1) * G, :],
                                 v[:, g * G:(g + 1) * G, :], ACTF.Silu)

    xp_state = {}

    def emit_xproj_h1(ch):
        psx = xp_ps.tile([NPJ, TC], FP32, tag="psx")
        xp_state["psx"] = psx
        for k in range(NDH // 2):
            nc.tensor.matmul(psx[:], xprj_sb[:, k * NPJ:(k + 1) * NPJ],
                             xiT[:, k * TC:(k + 1) * TC],
                             start=(k == 0), stop=False)

    def emit_xproj_h2(ch):
        psx = xp_state["psx"]
        for k in range(NDH // 2, NDH):
            nc.tensor.matmul(psx[:], xprj_sb[:, k * NPJ:(k + 1) * NPJ],
                             xiT[:, k * TC:(k + 1) * TC],
                             start=False, stop=(k == NDH - 1))
        dtT = bat_p.tile([DTR, TC], BF16, tag="dtT")
        nc.scalar.copy(dtT[:], psx[0:DTR, :])
        bc_sb = bat_p.tile([2 * DS, TC], BF16, tag="bc")
        nc.scalar.copy(bc_sb[:], psx[DTR:NPJ, :])
        nc.sync.dma_start(bc_d[ch % 2][:, :], bc_sb[:])
        return dtT

    def emit_bc_bcast(ch):
        # broadcast B/C rows for chunk ch into b_big/c_big with two wide
        # DMAs (DRAM source reshaped to one row, partition-broadcast)
        src = bc_d[ch % 2].ap().rearrange("(o s) t -> o (s t)", o=1)
        nc.scalar.dma_start(
            b_big[:].rearrange("p s t -> p (s t)"),
            src[0:1, 0:DS * TC].partition_broadcast(P))
        nc.scalar.dma_start(
            c_big[:].rearrange("p s t -> p (s t)"),
            src[0:1, DS * TC:2 * DS * TC].partition_broadcast(P))

    def emit_dt_all(ch, dtT):
        # dt_proj + softplus(x) = ln(1 + exp(x)), batched by function so
        # the ACT table loads once per phase. exp staging lives in u_big0
        # (dead between the last hc read of chunk ch-1 and u-mul of k=0).
        stage = u_big[0]
        for k in range(NDH):
            psd = mm_ps.tile([P, TC], FP32, tag="mm")
            nc.tensor.matmul(psd[:], dtw_sb[:, k * P:(k + 1) * P], dtT[:],
                             start=True, stop=True)
            nc.scalar.activation(stage[:, k, :], psd[:], ACTF.Exp,
                                 bias=dt_b_c(k))
        for g in range(4):
            nc.scalar.activation(
                delta[:].rearrange("p (kk t) -> p kk t", t=TC)
                [:, 4 * g:4 * g + 4, :],
                stage[:, 4 * g:4 * g + 4, :], ACTF.Ln, bias=1.0)

    def emit_z_group(ch, g, n=4):
        for m in range(4 * g, 4 * g + n):
            emit_in_proj_z(ch, m)

    pend = {}  # deferred ygate: k -> (yp tile, ksl)

    def emit_pend_gate():
        if "y" in pend:
            ypp, pksl = pend.pop("y")
            nc.vector.tensor_mul(ygate[:, pksl], ypp[:], z_sb[:, pksl])

    SD = DS - S_POOL  # states whose u/hc run on DVE; rest on Pool

    def emit_scan_k(ch, k):
        ksl = slice(k * TC, (k + 1) * TC)
        ub = u_big[k % 2]
        dx = t16_p.tile([P, TC], BF16, tag="dx")
        dx_eng.tensor_mul(dx[:], delta[:, ksl], xiT[:, ksl])
        dxv = dx[:].rearrange("p (o t) -> p o t", o=1)
        nc.vector.tensor_mul(ub[:, 0:SD, :],
                             dxv.broadcast_to([P, SD, TC]),
                             b_big[:, 0:SD, :])
        if S_POOL:
            nc.gpsimd.tensor_mul(ub[:, SD:DS, :],
                                 dxv.broadcast_to([P, S_POOL, TC]),
                                 b_big[:, SD:DS, :])
        yp = y_ps_p.tile([P, TC], FP32, tag="y")

        def scan_one(sj):
            av = sc_p.tile([P, TC], FP32, tag="a")
            nc.scalar.activation(av[:], delta[:, ksl], ACTF.Exp,
                                 scale=a_sb[:, k * DS + sj:k * DS + sj + 1])
            init = 0.0 if ch == 0 else carry[:, k * DS + sj:k * DS + sj + 1]
            nc.vector.tensor_tensor_scan(ub[:, sj, :], av[:], ub[:, sj, :],
                                         init, op0=OP.mult, op1=OP.add)

        nxt = ch + 1 < NCH
        for sj in range(SD):
            scan_one(sj)
        # DVE half: carry slice, then hc in place, PE accums start early
        if nxt:
            nc.vector.tensor_copy(carry[:, k * DS:k * DS + SD],
                                  ub[:, 0:SD, TC - 1])
        nc.vector.tensor_mul(ub[:, 0:SD, :], ub[:, 0:SD, :], c_big[:, 0:SD, :])
        nc.tensor.matmul(yp[:], dD_sb[:, k * P:(k + 1) * P],
                         xiT[:, ksl], start=True, stop=False)
        for sj in range(SD):
            nc.tensor.matmul(yp[:], ident[:], ub[:, sj, :],
                             start=False, stop=False)
        for sj in range(SD, DS):
            scan_one(sj)
        # previous k's gate: its PSUM result is ready by now, so this does
        # not head-of-line-block the DVE queue
        emit_pend_gate()
        if S_POOL:
            if nxt:
                nc.vector.tensor_copy(carry[:, k * DS + SD:(k + 1) * DS],
                                      ub[:, SD:DS, TC - 1])
            nc.gpsimd.tensor_mul(ub[:, SD:DS, :], ub[:, SD:DS, :],
                                 c_big[:, SD:DS, :])
        for sj in range(SD, DS):
            nc.tensor.matmul(yp[:], ident[:], ub[:, sj, :],
                             start=False, stop=(sj == DS - 1))
        pend["y"] = (yp, ksl)

    # ---------------- initial prep for chunk 0 ----------------
    emit_ln_apply(0)
    for m in range(NDH):
        emit_in_proj_xi(0, m)
    for k in range(NDH):
        emit_conv(0, k)
    emit_xi_silu(0)
    for m in range(NDH):
        emit_in_proj_z(0, m)
    emit_xproj_h1(0)
    dtT0 = emit_xproj_h2(0)
    emit_bc_bcast(0)
    emit_dt_all(0, dtT0)
    emit_ln_stats(list(range(TC // P, NLT)))  # remaining LN stats

    for ch in range(NCH):
        tsl = slice(ch * TC, (ch + 1) * TC)
        nxt = ch + 1 < NCH

        for k in range(NDH):
            # ---- interleaved prep for chunk ch+1 ----
            # WAR-safe schedule: every write to a tile region of ch+1 is
            # emitted only after scan(ch)'s last read of that region.
            if nxt:
                if k == 0:
                    emit_ln_apply(ch + 1)
                if 1 <= k <= 8:   # xz_xi windows (free since prep(ch))
                    emit_in_proj_xi(ch + 1, 2 * (k - 1))
                    emit_in_proj_xi(ch + 1, 2 * (k - 1) + 1)
                if 1 <= k <= 15:  # xiT[k-1]: scan(ch) k-1 reads are emitted
                    emit_conv(ch + 1, k - 1)
                if k == 10:  # silu cluster 1 + z_sb[0:8]; x_proj half 1
                    emit_xi_silu(ch + 1, gs=(0, 1))
                    emit_z_group(ch + 1, 0, n=8)
                    emit_xproj_h1(ch + 1)

            emit_scan_k(ch, k)

        emit_pend_gate()  # last k's gate
        if nxt:
            # tail of prep(ch+1): last conv, silu cluster 2, x_proj, dt
            emit_conv(ch + 1, 15)
            emit_xi_silu(ch + 1, gs=(2, 3))
            emit_z_group(ch + 1, 2, n=8)
            dtT = emit_xproj_h2(ch + 1)
            emit_bc_bcast(ch + 1)
            emit_dt_all(ch + 1, dtT)

        # ---- fused output projection
        for m in range(NDM):
            wt = wo_p.tile([P, NDH * P], BF16, tag="w_out")
            nc.sync.dma_start(
                wt[:], io["w_comb_pk"][:, m * NDH * P:(m + 1) * NDH * P])
            po = o_ps_p.tile([P, TC], FP32, tag="o")
            for k in range(NDH):
                nc.tensor.matmul(po[:], wt[:, k * P:(k + 1) * P],
                                 ygate[:, k * TC:(k + 1) * TC],
                                 start=(k == 0), stop=(k == NDH - 1))
            ot = ov_p.tile([P, TC], FP32, tag="out")
            nc.scalar.copy(ot[:], po[:])
            nc.scalar.dma_start(out_d[m * P:(m + 1) * P, tsl], ot[:])


# ------------------------------------------------------------------
# host side
# ------------------------------------------------------------------

def _prep_core_inputs(cfg, xb, w):
    """xb: [L, DM] fp32 (already flipped for bwd). w: per-branch weights dict."""
    d = {
        "x_ld": np.ascontiguousarray(xb, np.float32),
        "x_dl": np.ascontiguousarray(xb.T, np.float32),
    }
    d.update(w)
    return d


def _prep_branch_weights(cfg, in_w, conv_w, conv_b, xproj_w, dt_w, dt_b,
                         A_log, D, out_w, merge_half, ln_g, ln_b):
    w_comb = merge_half.astype(np.float64) @ out_w.astype(np.float64)  # [DM, DI]
    P = cfg.P

    def pack_lhsT(w):  # w: [M, K] -> [P, (M//P)*K]; block m holds w[mP:(m+1)P].T
        M, Kd = w.shape
        blocks = [w[m * P:(m + 1) * P, :].reshape(P, Kd // P, P)
                  .transpose(2, 1, 0).reshape(P, Kd)
                  for m in range(M // P)]
        return np.ascontiguousarray(np.concatenate(blocks, axis=1), BF16_NP)

    cw = np.asarray(conv_w, np.float32)
    DI, DC = cw.shape
    dg = np.zeros((DI // P, DC, P, P), np.float32)
    idx = np.arange(P)
    for k in range(DI // P):
        for t in range(DC):
            dg[k, t, idx, idx] = cw[k * P:(k + 1) * P, t]
    dg_pk = np.ascontiguousarray(
        dg.transpose(2, 0, 1, 3).reshape(P, (DI // P) * DC * P), BF16_NP)

    Dv = np.asarray(D, np.float32)
    dd = np.zeros((DI // P, P, P), np.float32)
    for k in range(DI // P):
        dd[k, idx, idx] = Dv[k * P:(k + 1) * P]
    dd_pk = np.ascontiguousarray(
        dd.transpose(1, 0, 2).reshape(P, (DI // P) * P), BF16_NP)

    in_w_g = np.asarray(in_w, np.float64) * np.asarray(ln_g, np.float64)[None, :]
    in_b = np.asarray(in_w, np.float64) @ np.asarray(ln_b, np.float64)

    return {
        "in_w_pk": pack_lhsT(in_w_g.astype(np.float32)),
        "in_b": np.ascontiguousarray(in_b.reshape(-1, 1), np.float32),
        "conv_dg_pk": dg_pk,
        "conv_b": np.ascontiguousarray(conv_b.reshape(-1, 1), np.float32),
        "xproj_wT": np.ascontiguousarray(xproj_w.T, BF16_NP),
        "dt_wT": np.ascontiguousarray(dt_w.T, BF16_NP),
        "dt_b": np.ascontiguousarray(dt_b.reshape(-1, 1), np.float32),
        "A_neg": np.ascontiguousarray(-np.exp(A_log), np.float32),
        "dD_pk": dd_pk,
        "w_comb_pk": pack_lhsT(w_comb.astype(np.float32)),
    }


_PROG_CACHE = {}


def _get_program(cfg: Cfg, num_devices: int):
    key = (cfg.L, cfg.DM, cfg.DI, cfg.DS, cfg.DTR, cfg.DC, cfg.TC, num_devices)
    if key not in _PROG_CACHE:
        _PROG_CACHE[key] = build_program(cfg, num_devices)
    return _PROG_CACHE[key]


def kernel(x, ln_g, ln_b, merge_w, merge_b,
           fwd_in_w, fwd_conv_w, fwd_conv_b, fwd_xproj_w, fwd_dt_w, fwd_dt_b,
           fwd_A_log, fwd_D, fwd_out_w,
           bwd_in_w, bwd_conv_w, bwd_conv_b, bwd_xproj_w, bwd_dt_w, bwd_dt_b,
           bwd_A_log, bwd_D, bwd_out_w):
    cfg = FULL
    x = np.asarray(x, np.float32)
    B = x.shape[0]
    assert x.shape == (B, cfg.L, cfg.DM) and B == 4

    nc = _get_program(cfg, 8)

    fw = _prep_branch_weights(cfg, fwd_in_w, fwd_conv_w, fwd_conv_b,
                              fwd_xproj_w, fwd_dt_w, fwd_dt_b, fwd_A_log,
                              fwd_D, fwd_out_w, np.asarray(merge_w)[:, :cfg.DM],
                              np.asarray(ln_g), np.asarray(ln_b))
    bw = _prep_branch_weights(cfg, bwd_in_w, bwd_conv_w, bwd_conv_b,
                              bwd_xproj_w, bwd_dt_w, bwd_dt_b, bwd_A_log,
                              bwd_D, bwd_out_w, np.asarray(merge_w)[:, cfg.DM:],
                              np.asarray(ln_g), np.asarray(ln_b))

    in_maps = []
    for c in range(8):
        br, b = divmod(c, 4)
        xb = x[b] if br == 0 else x[b, ::-1]
        in_maps.append(_prep_core_inputs(cfg, xb, fw if br == 0 else bw))

    global _last_in_maps
    _last_in_maps = in_maps
    res = run_bass_kernel_spmd(nc, in_maps, list(range(8)))
    parts = [r["part_out"] for r in res.results]  # [DM, L] each

    out = x.copy()
    for b in range(4):
        out[b] += parts[b].T
        out[b] += parts[4 + b].T[::-1]
    out += np.asarray(merge_b, np.float32)
    return out


# revision 43
# speedup vs baseline: 1.0888x; 1.0888x over previous
"""Bidirectional Mamba block on 8 Trainium2 NeuronCores.

Sharding: core c in 0..7 handles (branch = c // 4, batch = c % 4) where
branch 0 = fwd, branch 1 = bwd (bwd runs on the time-flipped input; flip
is applied host-side before dispatch and on the partial output after).

Per-core device pipeline (one full mamba branch for one batch element):
  LN -> in_proj (PE, bf16, ln-gamma folded into W) -> causal depthwise
  conv (PE, diag matmuls) -> silu -> x_proj (PE) -> dt_proj + softplus
  (exp/ln) -> selective scan: u = dx*B via one broadcast multiply,
  per-state in-place tensor_tensor_scan, hc = h*C in one multiply
  (split DVE/Pool by k), y = sum_s hC_s via PE identity-matmul PSUM
  accumulation seeded with diag(D)*xi -> gate with resident silu(z)
  -> fused (merge_half @ out_w) matmul -> partial output [d_model, L].

Host combines: out = x + part_fwd^T + flip(part_bwd^T) + merge_b.
"""

import math
import os
import sys
from contextlib import ExitStack

import numpy as np

sys.path.insert(0, "/opt/trn_rl_repo")
sys.path.insert(0, "/opt/trn_rl_repo/concourse")

import ml_dtypes  # noqa: E402

import concourse.bass as bass  # noqa: E402
import concourse.tile as tile  # noqa: E402
from concourse import bacc, mybir  # noqa: E402
from concourse.bass_utils import run_bass_kernel_spmd  # noqa: E402
from concourse.masks import make_identity  # noqa: E402

FP32 = mybir.dt.float32
BF16 = mybir.dt.bfloat16
OP = mybir.AluOpType
ACTF = mybir.ActivationFunctionType
BF16_NP = ml_dtypes.bfloat16

# ---- balance knobs (tuned against TimelineSim) ----
S_POOL = 5         # states whose u-mul/hc-mul run on gpsimd (rest DVE)
LN_ON_POOL = False  # LayerNorm apply (sub/mul) on gpsimd
DX_ON_POOL = False


class Cfg:
    def __init__(self, L=2048, DM=1024, DI=2048, DS=16, DTR=64, DC=4, TC=512):
        self.L = L      # sequence length
        self.DM = DM    # d_model
        self.DI = DI    # d_inner
        self.DS = DS    # d_state
        self.DTR = DTR  # dt_rank
        self.DC = DC    # d_conv
        self.TC = TC    # time chunk
        self.P = 128
        self.NCH = L // TC          # time chunks
        self.NDH = DI // self.P     # d_inner 128-tiles
        self.NDM = DM // self.P     # d_model 128-tiles
        self.NLT = L // self.P      # L 128-tiles (for LN stats)
        assert L % TC == 0 and DI % 128 == 0 and DM % 128 == 0 and L % 128 == 0
        assert DTR <= 128 and DTR + 2 * DS <= 128


FULL = Cfg()


def build_program(cfg: Cfg, num_devices: int = 8):
    """Build the (shared-across-cores) Bass program."""
    nc = bacc.Bacc(
        "TRN2", target_bir_lowering=False, debug=False, num_devices=num_devices
    )
    P, L = cfg.P, cfg.L

    def ext_in(name, shape, dt=FP32):
        return nc.dram_tensor(name, shape, dt, kind="ExternalInput")

    io = {
        # activations
        "x_ld": ext_in("x_ld", [L, cfg.DM]),          # [L, d_model] fp32
        "x_dl": ext_in("x_dl", [cfg.DM, L]),          # transposed   fp32
        # weights (pre-transposed / pre-cast host side)
        "in_w_pk": ext_in("in_w_pk", [P, 2 * (cfg.DI // P) * cfg.DM], BF16),
        "in_b": ext_in("in_b", [2 * cfg.DI, 1]),      # in_w @ ln_b
        "conv_dg_pk": ext_in(
            "conv_dg_pk", [P, (cfg.DI // P) * cfg.DC * P], BF16),
        "conv_b": ext_in("conv_b", [cfg.DI, 1]),
        "xproj_wT": ext_in("xproj_wT", [cfg.DI, cfg.DTR + 2 * cfg.DS], BF16),
        "dt_wT": ext_in("dt_wT", [cfg.DTR, cfg.DI], BF16),
        "dt_b": ext_in("dt_b", [cfg.DI, 1]),
        "A_neg": ext_in("A_neg", [cfg.DI, cfg.DS]),   # -exp(A_log) fp32
        "D_vec": ext_in("D_vec", [cfg.DI, 1]),
        "w_comb_pk": ext_in("w_comb_pk", [P, (cfg.DM // P) * cfg.DI], BF16),
    }
    out = nc.dram_tensor("part_out", [cfg.DM, L], FP32, kind="ExternalOutput")
    # internal DRAM scratch
    scratch = {
        "mu_d": [nc.dram_tensor(f"mu_d{c}", [cfg.TC, 1], BF16)
                 for c in range(cfg.NCH)],
        "rstd_d": [nc.dram_tensor(f"rstd_d{c}", [cfg.TC, 1], BF16)
                   for c in range(cfg.NCH)],
        "bc_d": [nc.dram_tensor(f"bc_d{j}", [2 * cfg.DS, cfg.TC], BF16)
                 for j in range(2)],
    }

    with tile.TileContext(nc) as tc:
        with ExitStack() as ctx:
            _body(ctx, tc, cfg, io, out, scratch)
    nc.compile()
    return nc


def _body(ctx, tc, cfg, io, out_d, scratch):
    nc = tc.nc
    P, L, TC, DS, DC = cfg.P, cfg.L, cfg.TC, cfg.DS, cfg.DC
    NCH, NDH, NDM = cfg.NCH, cfg.NDH, cfg.NDM
    NLT, DTR = cfg.NLT, cfg.DTR
    CW = TC + DC - 1  # conv input window per chunk in the xz store
    NPJ = DTR + 2 * DS
    mu_d, rstd_d, bc_d = scratch["mu_d"], scratch["rstd_d"], scratch["bc_d"]

    # ---------------- persistent pools / tiles ----------------
    const_p = ctx.enter_context(tc.tile_pool(name="const", bufs=1))
    big_p = ctx.enter_context(tc.tile_pool(name="big", bufs=1))

    ident = const_p.tile([P, P], BF16, tag="ident")
    make_identity(nc, ident[:])

    # small per-channel columns packed into one tile:
    # [NDH conv_b][NDH dt_b][2*NDH in_b][1 eps]
    ncc = 4 * NDH + 1
    cols = const_p.tile([P, ncc], FP32, tag="cols")
    o_cb, o_db, o_ib = 0, NDH, 2 * NDH
    o_eps = 4 * NDH
    conv_b_c = lambda k: cols[:, o_cb + k:o_cb + k + 1]
    dt_b_c = lambda k: cols[:, o_db + k:o_db + k + 1]
    in_b_c = lambda m: cols[:, o_ib + m:o_ib + m + 1]
    eps_c = cols[:, o_eps:o_eps + 1]
    nc.vector.memset(eps_c, 1e-5)
    for k in range(NDH):
        r = slice(k * P, (k + 1) * P)
        nc.sync.dma_start(conv_b_c(k), io["conv_b"][r, :])
        nc.sync.dma_start(dt_b_c(k), io["dt_b"][r, :])
    for m in range(2 * NDH):
        r = slice(m * P, (m + 1) * P)
        nc.sync.dma_start(in_b_c(m), io["in_b"][r, :])

    # ---------------- phase 1: LayerNorm statistics ----------------
    # Batched by activation function so the ACT table never ping-pongs:
    # per-tile Square+accum, then batched Ln / Exp. Emitted in two groups
    # so chunk-0 prep can start as soon as its own stats exist.
    ln_p = ctx.enter_context(tc.tile_pool(name="ln", bufs=1))
    lns_p = ctx.enter_context(tc.tile_pool(name="lns", bufs=1))
    mu_all = lns_p.tile([P, NLT], BF16, tag="mu_all")
    ss_all = lns_p.tile([P, NLT], FP32, tag="ss_all")
    lv_all = lns_p.tile([P, NLT], FP32, tag="lv_all")
    rstd_all = lns_p.tile([P, NLT], BF16, tag="rstd_all")

    def emit_ln_stats(lts):
        for lt in lts:
            r = slice(lt * P, (lt + 1) * P)
            xt = ln_p.tile([P, cfg.DM], FP32, tag="x")
            nc.scalar.dma_start(xt[:], io["x_ld"][r, :])
            s1 = ln_p.tile([P, 1], FP32, tag="s1")
            nc.vector.reduce_sum(s1[:], xt[:], axis=mybir.AxisListType.X)
            negmu = ln_p.tile([P, 1], FP32, tag="negmu")
            nc.scalar.mul(negmu[:], s1[:], -1.0 / cfg.DM)
            nc.scalar.mul(mu_all[:, lt:lt + 1], s1[:], 1.0 / cfg.DM)
            sq = ln_p.tile([P, cfg.DM], BF16, tag="sq")
            nc.scalar.activation(sq[:], xt[:], ACTF.Square, bias=negmu[:],
                                 scale=1.0, accum_out=ss_all[:, lt:lt + 1])
        # rstd = exp(-0.5 * ln(var + eps)); keeps ACT in the exp/ln table
        ls = slice(lts[0], lts[-1] + 1)
        nc.scalar.activation(lv_all[:, ls], ss_all[:, ls], ACTF.Ln, bias=eps_c,
                             scale=1.0 / cfg.DM)
        nc.scalar.activation(rstd_all[:, ls], lv_all[:, ls], ACTF.Exp,
                             scale=-0.5)
        for lt in lts:
            cix = (lt * P) // TC
            roff = lt * P - cix * TC
            nc.sync.dma_start(mu_d[cix][roff:roff + P, :], mu_all[:, lt:lt + 1])
            nc.sync.dma_start(rstd_d[cix][roff:roff + P, :],
                              rstd_all[:, lt:lt + 1])

    emit_ln_stats(list(range(TC // P)))  # chunk 0's stats first

    a_sb = const_p.tile([P, NDH * DS], FP32, tag="aneg")
    for k in range(NDH):
        nc.sync.dma_start(a_sb[:, k * DS:(k + 1) * DS],
                          io["A_neg"][k * P:(k + 1) * P, :])

    # x_proj / dt_proj weights resident, bf16
    xprj_sb = const_p.tile([P, NDH * NPJ], BF16, tag="xprj")
    for k in range(NDH):
        nc.sync.dma_start(
            xprj_sb[:, k * NPJ:(k + 1) * NPJ], io["xproj_wT"][k * P:(k + 1) * P, :]
        )
    dtw_sb = const_p.tile([DTR, cfg.DI], BF16, tag="dtw")
    nc.sync.dma_start(dtw_sb[:], io["dt_wT"][:, :])
    dvec = const_p.tile([P, NDH], FP32, tag="dvec")
    for k in range(NDH):
        nc.sync.dma_start(dvec[:, k:k + 1], io["D_vec"][k * P:(k + 1) * P, :])

    # ---------------- persistent chunk-state tiles ----------------
    xz_xi = big_p.tile([P, NDH * CW], BF16, tag="xz_xi")
    xiT = big_p.tile([P, NDH * TC], BF16, tag="xiT")
    delta = big_p.tile([P, NDH * TC], BF16, tag="delta")
    z_sb = big_p.tile([P, NDH * TC], BF16, tag="z_sb")
    b_big = big_p.tile([P, DS, TC], BF16, tag="b_big")
    c_big = big_p.tile([P, DS, TC], BF16, tag="c_big")
    u_big0 = big_p.tile([P, DS, TC], BF16, tag="u_big0")
    u_big1 = big_p.tile([P, DS, TC], BF16, tag="u_big1")
    u_big = [u_big0, u_big1]
    ygate = big_p.tile([P, NDH * TC], BF16, tag="ygate")
    carry = big_p.tile([P, NDH * DS], FP32, tag="carry")
    xnT = big_p.tile([P, NDM * TC], BF16, tag="xnT")

    for k in range(NDH):  # zero the conv left-pad for chunk 0
        nc.vector.memset(xz_xi[:, k * CW:k * CW + DC - 1], 0.0)

    wi_p = ctx.enter_context(tc.tile_pool(name="wi", bufs=3))
    wo_p = ctx.enter_context(tc.tile_pool(name="wo", bufs=2))
    mm_ps = ctx.enter_context(
        tc.tile_pool(name="mmps", bufs=3, space=bass.MemorySpace.PSUM))
    y_ps_p = ctx.enter_context(
        tc.tile_pool(name="yps", bufs=3, space=bass.MemorySpace.PSUM))
    xp_ps = ctx.enter_context(
        tc.tile_pool(name="xpps", bufs=1, space=bass.MemorySpace.PSUM))
    o_ps_p = ctx.enter_context(
        tc.tile_pool(name="ops", bufs=1, space=bass.MemorySpace.PSUM))
    sc_p = ctx.enter_context(tc.tile_pool(name="scan", bufs=4))
    t16_p = ctx.enter_context(tc.tile_pool(name="t16", bufs=2))
    ev_p = ctx.enter_context(tc.tile_pool(name="evac", bufs=2))
    ov_p = ctx.enter_context(tc.tile_pool(name="oev", bufs=1))
    bat_p = ctx.enter_context(tc.tile_pool(name="bat", bufs=1))
    lnc_p = ctx.enter_context(tc.tile_pool(name="lnc", bufs=1))

    ln_eng = nc.gpsimd if LN_ON_POOL else nc.vector
    dx_eng = nc.gpsimd if DX_ON_POOL else nc.vector

    def emit_ln_apply(ch):
        tsl = slice(ch * TC, (ch + 1) * TC)
        mus = lnc_p.tile([P, 2 * TC], BF16, tag="mus")
        nc.sync.dma_start(
            mus[:, 0:TC],
            mu_d[ch].ap().rearrange("l one -> one l")[0:1, :].partition_broadcast(P))
        nc.sync.dma_start(
            mus[:, TC:2 * TC],
            rstd_d[ch].ap().rearrange("l one -> one l")[0:1, :].partition_broadcast(P))
        for k in range(NDM):
            xtT = ev_p.tile([P, TC], FP32, tag="lnx")
            nc.sync.dma_start(xtT[:], io["x_dl"][k * P:(k + 1) * P, tsl])
            ln_eng.tensor_sub(xtT[:], xtT[:], mus[:, 0:TC])
            ln_eng.tensor_mul(xnT[:, k * TC:(k + 1) * TC], xtT[:],
                              mus[:, TC:2 * TC])

    def emit_in_proj_xi(ch, m):
        wt = wi_p.tile([P, NDM * P], BF16, tag="w_in")
        nc.sync.dma_start(
            wt[:], io["in_w_pk"][:, m * NDM * P:(m + 1) * NDM * P])
        ps = mm_ps.tile([P, TC], FP32, tag="mm")
        for k in range(NDM):
            nc.tensor.matmul(ps[:], wt[:, k * P:(k + 1) * P],
                             xnT[:, k * TC:(k + 1) * TC],
                             start=(k == 0), stop=(k == NDM - 1))
        nc.scalar.activation(xz_xi[:, m * CW + DC - 1:m * CW + DC - 1 + TC],
                             ps[:], ACTF.Identity, bias=in_b_c(m))

    def emit_in_proj_z(ch, m):
        # m in [0, NDH): z-half tile; direct Silu evac into z_sb
        wt = wi_p.tile([P, NDM * P], BF16, tag="w_in")
        nc.sync.dma_start(
            wt[:], io["in_w_pk"][:, (NDH + m) * NDM * P:(NDH + m + 1) * NDM * P])
        ps = mm_ps.tile([P, TC], FP32, tag="mm")
        for k in range(NDM):
            nc.tensor.matmul(ps[:], wt[:, k * P:(k + 1) * P],
                             xnT[:, k * TC:(k + 1) * TC],
                             start=(k == 0), stop=(k == NDM - 1))
        nc.scalar.activation(z_sb[:, m * TC:(m + 1) * TC], ps[:],
                             ACTF.Silu, bias=in_b_c(NDH + m))

    def emit_conv(ch, k):
        # matmuls + raw (Identity+bias) evac into xiT; silu applied later
        dg = wi_p.tile([P, DC * P], BF16, tag="w_dg")
        nc.sync.dma_start(
            dg[:], io["conv_dg_pk"][:, k * DC * P:(k + 1) * DC * P])
        ps = mm_ps.tile([P, TC], FP32, tag="mm")
        for t in range(DC):
            nc.tensor.matmul(
                ps[:], dg[:, t * P:(t + 1) * P],
                xz_xi[:, k * CW + t:k * CW + t + TC],
                start=(t == 0), stop=(t == DC - 1))
        nc.scalar.activation(xiT[:, k * TC:(k + 1) * TC], ps[:],
                             ACTF.Identity, bias=conv_b_c(k))
        if ch + 1 < NCH:  # carry last DC-1 input cols for the next chunk
            nc.vector.tensor_copy(xz_xi[:, k * CW:k * CW + DC - 1],
                                  xz_xi[:, k * CW + TC:k * CW + TC + DC - 1])

    def emit_xi_silu(ch, gs=(0, 1, 2, 3)):
        # in-place Silu over xiT, 4-tile-wide ops
        G = 4
        v = xiT[:].rearrange("p (kk t) -> p kk t", t=TC)
        for g in gs:
            nc.scalar.activation(v[:, g * G:(g + 1) * G, :],
                                 v[:, g * G:(g + 1) * G, :], ACTF.Silu)

    xp_state = {}

    def emit_xproj_h1(ch):
        psx = xp_ps.tile([NPJ, TC], FP32, tag="psx")
        xp_state["psx"] = psx
        for k in range(NDH // 2):
            nc.tensor.matmul(psx[:], xprj_sb[:, k * NPJ:(k + 1) * NPJ],
                             xiT[:, k * TC:(k + 1) * TC],
                             start=(k == 0), stop=False)

    def emit_xproj_h2(ch):
        psx = xp_state["psx"]
        for k in range(NDH // 2, NDH):
            nc.tensor.matmul(psx[:], xprj_sb[:, k * NPJ:(k + 1) * NPJ],
                             xiT[:, k * TC:(k + 1) * TC],
                             start=False, stop=(k == NDH - 1))
        dtT = bat_p.tile([DTR, TC], BF16, tag="dtT")
        nc.scalar.copy(dtT[:], psx[0:DTR, :])
        bc_sb = bat_p.tile([2 * DS, TC], BF16, tag="bc")
        nc.scalar.copy(bc_sb[:], psx[DTR:NPJ, :])
        nc.sync.dma_start(bc_d[ch % 2][:, :], bc_sb[:])
        return dtT

    def emit_bc_bcast(ch):
        # broadcast B/C rows for chunk ch into b_big/c_big with two wide
        # DMAs (DRAM source reshaped to one row, partition-broadcast)
        src = bc_d[ch % 2].ap().rearrange("(o s) t -> o (s t)", o=1)
        nc.scalar.dma_start(
            b_big[:].rearrange("p s t -> p (s t)"),
            src[0:1, 0:DS * TC].partition_broadcast(P))
        nc.scalar.dma_start(
            c_big[:].rearrange("p s t -> p (s t)"),
            src[0:1, DS * TC:2 * DS * TC].partition_broadcast(P))

    def emit_dt_all(ch, dtT):
        # dt_proj + softplus(x) = ln(1 + exp(x)), batched by function so
        # the ACT table loads once per phase. exp staging lives in u_big0
        # (dead between the last hc read of chunk ch-1 and u-mul of k=0).
        stage = u_big[0]
        for k in range(NDH):
            psd = mm_ps.tile([P, TC], FP32, tag="mm")
            nc.tensor.matmul(psd[:], dtw_sb[:, k * P:(k + 1) * P], dtT[:],
                             start=True, stop=True)
            nc.scalar.activation(stage[:, k, :], psd[:], ACTF.Exp,
                                 bias=dt_b_c(k))
        for g in range(4):
            nc.scalar.activation(
                delta[:].rearrange("p (kk t) -> p kk t", t=TC)
                [:, 4 * g:4 * g + 4, :],
                stage[:, 4 * g:4 * g + 4, :], ACTF.Ln, bias=1.0)

    def emit_z_group(ch, g, n=4):
        for m in range(4 * g, 4 * g + n):
            emit_in_proj_z(ch, m)

    pend = {}  # deferred ygate: k -> (yp tile, ksl)

    def emit_pend_gate():
        if "y" in pend:
            ypp, pksl = pend.pop("y")
            nc.vector.tensor_mul(ygate[:, pksl], ypp[:], z_sb[:, pksl])

    SD = DS - S_POOL  # states whose u/hc run on DVE; rest on Pool

    def emit_scan_k(ch, k):
        ksl = slice(k * TC, (k + 1) * TC)
        ub = u_big[k % 2]
        dx = t16_p.tile([P, TC], BF16, tag="dx")
        dx_eng.tensor_mul(dx[:], delta[:, ksl], xiT[:, ksl])
        dxv = dx[:].rearrange("p (o t) -> p o t", o=1)
        nc.vector.tensor_mul(ub[:, 0:SD, :],
                             dxv.broadcast_to([P, SD, TC]),
                             b_big[:, 0:SD, :])
        if S_POOL:
            nc.gpsimd.tensor_mul(ub[:, SD:DS, :],
                                 dxv.broadcast_to([P, S_POOL, TC]),
                                 b_big[:, SD:DS, :])
        yp = y_ps_p.tile([P, TC], FP32, tag="y")

        def scan_one(sj):
            av = sc_p.tile([P, TC], FP32, tag="a")
            nc.scalar.activation(av[:], delta[:, ksl], ACTF.Exp,
                                 scale=a_sb[:, k * DS + sj:k * DS + sj + 1])
            init = 0.0 if ch == 0 else carry[:, k * DS + sj:k * DS + sj + 1]
            nc.vector.tensor_tensor_scan(ub[:, sj, :], av[:], ub[:, sj, :],
                                         init, op0=OP.mult, op1=OP.add)

        nxt = ch + 1 < NCH
        for sj in range(SD):
            scan_one(sj)
        # DVE half: carry slice, then hc in place, PE accums start early
        if nxt:
            nc.vector.tensor_copy(carry[:, k * DS:k * DS + SD],
                                  ub[:, 0:SD, TC - 1])
        xid = t16_p.tile([P, TC], BF16, tag="xid")
        nc.vector.tensor_scalar_mul(xid[:], xiT[:, ksl], dvec[:, k:k + 1])
        nc.vector.tensor_mul(ub[:, 0:SD, :], ub[:, 0:SD, :], c_big[:, 0:SD, :])
        nc.tensor.matmul(yp[:], ident[:], xid[:], start=True, stop=False)
        for sj in range(SD):
            nc.tensor.matmul(yp[:], ident[:], ub[:, sj, :],
                             start=False, stop=False)
        for sj in range(SD, DS):
            scan_one(sj)
        # previous k's gate: its PSUM result is ready by now, so this does
        # not head-of-line-block the DVE queue
        emit_pend_gate()
        if S_POOL:
            if nxt:
                nc.vector.tensor_copy(carry[:, k * DS + SD:(k + 1) * DS],
                                      ub[:, SD:DS, TC - 1])
            nc.gpsimd.tensor_mul(ub[:, SD:DS, :], ub[:, SD:DS, :],
                                 c_big[:, SD:DS, :])
        for sj in range(SD, DS):
            nc.tensor.matmul(yp[:], ident[:], ub[:, sj, :],
                             start=False, stop=(sj == DS - 1))
        pend["y"] = (yp, ksl)

    # ---------------- initial prep for chunk 0 ----------------
    emit_ln_apply(0)
    for m in range(NDH):
        emit_in_proj_xi(0, m)
    for k in range(NDH):
        emit_conv(0, k)
    emit_xi_silu(0)
    for m in range(NDH):
        emit_in_proj_z(0, m)
    emit_xproj_h1(0)
    dtT0 = emit_xproj_h2(0)
    emit_bc_bcast(0)
    emit_dt_all(0, dtT0)
    emit_ln_stats(list(range(TC // P, NLT)))  # remaining LN stats

    for ch in range(NCH):
        tsl = slice(ch * TC, (ch + 1) * TC)
        nxt = ch + 1 < NCH

        for k in range(NDH):
            # ---- interleaved prep for chunk ch+1 ----
            # WAR-safe schedule: every write to a tile region of ch+1 is
            # emitted only after scan(ch)'s last read of that region.
            if nxt:
                if k == 0:
                    emit_ln_apply(ch + 1)
                if 1 <= k <= 8:   # xz_xi windows (free since prep(ch))
                    emit_in_proj_xi(ch + 1, 2 * (k - 1))
                    emit_in_proj_xi(ch + 1, 2 * (k - 1) + 1)
                if 1 <= k <= 15:  # xiT[k-1]: scan(ch) k-1 reads are emitted
                    emit_conv(ch + 1, k - 1)
                if k == 10:  # silu cluster 1 + z_sb[0:8]; x_proj half 1
                    emit_xi_silu(ch + 1, gs=(0, 1))
                    emit_z_group(ch + 1, 0, n=8)
                    emit_xproj_h1(ch + 1)

            emit_scan_k(ch, k)

        emit_pend_gate()  # last k's gate
        if nxt:
            # tail of prep(ch+1): last conv, silu cluster 2, x_proj, dt
            emit_conv(ch + 1, 15)
            emit_xi_silu(ch + 1, gs=(2, 3))
            emit_z_group(ch + 1, 2, n=8)
            dtT = emit_xproj_h2(ch + 1)
            emit_bc_bcast(ch + 1)
            emit_dt_all(ch + 1, dtT)

        # ---- fused output projection
        for m in range(NDM):
            wt = wo_p.tile([P, NDH * P], BF16, tag="w_out")
            nc.sync.dma_start(
                wt[:], io["w_comb_pk"][:, m * NDH * P:(m + 1) * NDH * P])
            po = o_ps_p.tile([P, TC], FP32, tag="o")
            for k in range(NDH):
                nc.tensor.matmul(po[:], wt[:, k * P:(k + 1) * P],
                                 ygate[:, k * TC:(k + 1) * TC],
                                 start=(k == 0), stop=(k == NDH - 1))
            ot = ov_p.tile([P, TC], FP32, tag="out")
            nc.scalar.copy(ot[:], po[:])
            nc.scalar.dma_start(out_d[m * P:(m + 1) * P, tsl], ot[:])


# ------------------------------------------------------------------
# host side
# ------------------------------------------------------------------

def _prep_core_inputs(cfg, xb, w):
    """xb: [L, DM] fp32 (already flipped for bwd). w: per-branch weights dict."""
    d = {
        "x_ld": np.ascontiguousarray(xb, np.float32),
        "x_dl": np.ascontiguousarray(xb.T, np.float32),
    }
    d.update(w)
    return d


def _prep_branch_weights(cfg, in_w, conv_w, conv_b, xproj_w, dt_w, dt_b,
                         A_log, D, out_w, merge_half, ln_g, ln_b):
    w_comb = merge_half.astype(np.float64) @ out_w.astype(np.float64)  # [DM, DI]
    P = cfg.P

    def pack_lhsT(w):  # w: [M, K] -> [P, (M//P)*K]; block m holds w[mP:(m+1)P].T
        M, Kd = w.shape
        blocks = [w[m * P:(m + 1) * P, :].reshape(P, Kd // P, P)
                  .transpose(2, 1, 0).reshape(P, Kd)
                  for m in range(M // P)]
        return np.ascontiguousarray(np.concatenate(blocks, axis=1), BF16_NP)

    cw = np.asarray(conv_w, np.float32)
    DI, DC = cw.shape
    dg = np.zeros((DI // P, DC, P, P), np.float32)
    idx = np.arange(P)
    for k in range(DI // P):
        for t in range(DC):
            dg[k, t, idx, idx] = cw[k * P:(k + 1) * P, t]
    dg_pk = np.ascontiguousarray(
        dg.transpose(2, 0, 1, 3).reshape(P, (DI // P) * DC * P), BF16_NP)

    in_w_g = np.asarray(in_w, np.float64) * np.asarray(ln_g, np.float64)[None, :]
    in_b = np.asarray(in_w, np.float64) @ np.asarray(ln_b, np.float64)

    return {
        "in_w_pk": pack_lhsT(in_w_g.astype(np.float32)),
        "in_b": np.ascontiguousarray(in_b.reshape(-1, 1), np.float32),
        "conv_dg_pk": dg_pk,
        "conv_b": np.ascontiguousarray(conv_b.reshape(-1, 1), np.float32),
        "xproj_wT": np.ascontiguousarray(xproj_w.T, BF16_NP),
        "dt_wT": np.ascontiguousarray(dt_w.T, BF16_NP),
        "dt_b": np.ascontiguousarray(dt_b.reshape(-1, 1), np.float32),
        "A_neg": np.ascontiguousarray(-np.exp(A_log), np.float32),
        "D_vec": np.ascontiguousarray(np.asarray(D, np.float32).reshape(-1, 1)),
        "w_comb_pk": pack_lhsT(w_comb.astype(np.float32)),
    }


_PROG_CACHE = {}


def _get_program(cfg: Cfg, num_devices: int):
    key = (cfg.L, cfg.DM, cfg.DI, cfg.DS, cfg.DTR, cfg.DC, cfg.TC, num_devices)
    if key not in _PROG_CACHE:
        _PROG_CACHE[key] = build_program(cfg, num_devices)
    return _PROG_CACHE[key]


def kernel(x, ln_g, ln_b, merge_w, merge_b,
           fwd_in_w, fwd_conv_w, fwd_conv_b, fwd_xproj_w, fwd_dt_w, fwd_dt_b,
           fwd_A_log, fwd_D, fwd_out_w,
           bwd_in_w, bwd_conv_w, bwd_conv_b, bwd_xproj_w, bwd_dt_w, bwd_dt_b,
           bwd_A_log, bwd_D, bwd_out_w):
    cfg = FULL
    x = np.asarray(x, np.float32)
    B = x.shape[0]
    assert x.shape == (B, cfg.L, cfg.DM) and B == 4

    nc = _get_program(cfg, 8)

    fw = _prep_branch_weights(cfg, fwd_in_w, fwd_conv_w, fwd_conv_b,
                              fwd_xproj_w, fwd_dt_w, fwd_dt_b, fwd_A_log,
                              fwd_D, fwd_out_w, np.asarray(merge_w)[:, :cfg.DM],
                              np.asarray(ln_g), np.asarray(ln_b))
    bw = _prep_branch_weights(cfg, bwd_in_w, bwd_conv_w, bwd_conv_b,
                              bwd_xproj_w, bwd_dt_w, bwd_dt_b, bwd_A_log,
                              bwd_D, bwd_out_w, np.asarray(merge_w)[:, cfg.DM:],
                              np.asarray(ln_g), np.asarray(ln_b))

    in_maps = []
    for c in range(8):
        br, b = divmod(c, 4)
        xb = x[b] if br == 0 else x[b, ::-1]
        in_maps.append(_prep_core_inputs(cfg, xb, fw if br == 0 else bw))

    global _last_in_maps
    _last_in_maps = in_maps
    res = run_bass_kernel_spmd(nc, in_maps, list(range(8)))
    parts = [r["part_out"] for r in res.results]  # [DM, L] each

    out = x.copy()
    for b in range(4):
        out[b] += parts[b].T
        out[b] += parts[4 + b].T[::-1]
    out += np.asarray(merge_b, np.float32)
    return out
